# revision 71
# baseline (speedup 1.0000x reference)
"""Trainium2 Bass kernel for nn_DSC_28535762715377.

Computes u[c] = M_tilde[0,0] @ y_rev[0]
             + sum_ij  c2[i,j] (M_tilde[i,j] @ y_rev[j])
             + sum_lk  c3[l,k] (M[l,k,0,0] @ y_rev[k])
             + sum_ijlk c2[i,j] c3[l,k] (M[i,j,l,k] @ y_rev[j+k])

Term 3 streams the 340 MB M tensor; everything else is <1% of the bytes.
Strategy: shard M's leading i axis across 8 cores (3 i-values each,
42.5 MB contiguous per core). On each core, stream the slab through the
tensor engine as a weighted reduction: rhs tiles [128 part, 512 free]
(contiguous HBM), stationary lhsT [128, 64] holding the per-row weights
w[row,p'] = c2[i,j]*c3[l,k]*y_rev[j+k,p'] replicated over 8 p' columns
and 8 row-triples, accumulated into a single PSUM bank [64, 512].
The valid entries sit on the (triple, p'==p) diagonal; the host gathers
them and all-reduces over cores. Small terms 0-2 are computed on host.

Perf structure (fp8e3 mode, measured 53-59 us/core depending on chip
DVFS state, vs 123 us fp32r baseline):
- M quantized to fp8 e3m4 on host (1 B/elem, 10.6 MB/core): the 10.6M-
  term random sum keeps rel err at the per-element sigma (~1.3e-2 with
  bf16 weights + bf16 yc3 consts, deterministic; gate is 2e-2).
- PE streams 135 matmuls x 512 cols at 1 col/cycle @2.4 GHz = 29 us;
  DMA floor 11.1 MB at ~367 GB/s = 30 us — balanced when unthrottled.
- 3 of 18 chunks are offloaded to the otherwise-idle Vector engine as
  8 contiguous scalar_tensor_tensor fused multiply+reduce ops each
  (~760 ns per [128, 576] op; strided patterns are ~9x slower, hence
  the host-side c-major relayout of those chunks). Their [128, 1]
  partials cross-partition-sum via one ones-matmul into "out_d".
- All latency-critical DMA rides the gpsimd queue in explicit FIFO
  order (sync/scalar queues serve at only ~20-40 GB/s and steal port
  bandwidth if loaded): y0 slice, chunk0 (split so the first matmul
  gates on 1/3 of it), then chunks 1-15 as 3-wide groups whose
  partition lines are 13.8 KB (host-relaid; 4.6 KB lines only reach
  ~280 GB/s), chunks 16/17 single at the tail, remaining yc3 slices
  interleaved just ahead of their first consumer. All 5 triples stay
  resident (mg bufs=5) so the slow DVE consumption never blocks the
  DMA stream on a buffer slot.
- Dummy warmup matmuls (+4 fillers after chunk0) burn the PE DVFS ramp
  (~3 us at half clock after idle) on throwaway work.

MODE:
  "fp32"   — exact, PE-bound (fp32 matmul = 4 cycles/row).
  "fp32r"  — TF32-ish matmul (1 cycle/row), rel err ~1.5e-4.
  "bf16"   — M in bf16 (half the HBM bytes), bf16 weights, 1 pass.
             rel err ~6e-3 (harness gate is 2e-2).
  "fp8e3"  — M in fp8 e3m4 (quarter the HBM bytes), bf16 weights
             (mixed-dtype matmul), 1 pass. rel err ~1.1e-2,
             deterministic (fixed seed inputs).
  "bf16x2" — hi/lo bf16 split, 3 matmul passes (hi*hi, hi*lo, lo*hi).
             Same HBM bytes as fp32, rel err ~1e-5, PE 3 cycles/4B.
ONCHIP_W: generate weight tiles on-chip (DVE) from tiny factors instead
of streaming them fused with the M chunks (-11% HBM traffic).
"""

import numpy as np

# ---- problem constants (hardcoded; kernel.py must be self-contained) ----
H, MDIM, C, P = 24, 48, 8, 8
NCORES = 8
IPC = H // NCORES          # i-values per core = 3
NJC = 6                    # j-chunks per i
JCW = MDIM // NJC          # j per chunk = 8
ROWS = JCW * H * MDIM      # rows (of 64 floats) per chunk = 9216
RPP = ROWS // 128          # rows per partition = 72
NT = RPP // 8              # matmuls per chunk = 9
NCHUNK = IPC * NJC         # chunks per core = 18
WCOLS = RPP * 8            # 576 weight columns per chunk
MCOLS = RPP * 64           # 4608 data columns per chunk
NCC = NJC * WCOLS          # consts yc3 columns

MODE = "fp8e3"            # "fp32" | "fp32r" | "bf16" | "fp8e3" | "bf16x2"
ONCHIP_W = True
# chunks computed on the Vector engine via fused multiply+reduce instead
# of the PE, cutting the PE stream. These chunks are relaid c-major on
# host so each of the 8 c-slices is a contiguous [128, 576] block (the
# DVE is ~9x slower on strided access patterns). Their partials return
# via a tiny ones-matmul cross-partition sum in the "out_d" output.
DVE_CHUNKS = (2, 6, 10)

_prog_cache = {}


def _bf16(x):
    import ml_dtypes
    return np.asarray(x).astype(ml_dtypes.bfloat16)


def _build_program():
    import concourse.bass as bass
    import concourse.mybir as mybir
    from concourse.tile import TileContext

    fp32 = mybir.dt.float32
    bf16 = mybir.dt.bfloat16
    mmdt = {"fp32": fp32, "fp32r": mybir.dt.float32r, "bf16": bf16,
            "fp8e3": mybir.dt.float8e3, "bf16x2": bf16}[MODE]
    wdt = bf16 if MODE in ("bf16", "fp8e3") else mmdt
    nc = bass.Bass()

    # fused-chunk column layout (in mmdt elements)
    if MODE == "bf16x2":
        ccols_fused = 2 * WCOLS + 2 * MCOLS     # wh | wl | Mh | Ml
        m_cols = 2 * MCOLS                      # Mh | Ml (onchip variant)
    else:
        ccols_fused = WCOLS + MCOLS             # w | M
        m_cols = MCOLS

    fp16 = mybir.dt.float16
    if ONCHIP_W:
        # chunks jc-major, all M + yc3 traffic on the gpsimd queue in
        # explicit FIFO order (other queues are slow and steal port bw):
        # y0, c0 (split), c1, c2 singles for fast pipeline fill, then
        # triples with 13.8 KB partition lines (~367 GB/s), yc3 slices
        # interleaved just ahead of their first consumer.
        m_sing = nc.dram_tensor("m_sing", [3, 128, m_cols], mmdt,
                                kind="ExternalInput")
        m_trip = nc.dram_tensor("m_trip", [5, 128, IPC * m_cols], mmdt,
                                kind="ExternalInput")
        consts_a = nc.dram_tensor("consts_a", [128, NCHUNK], fp32,
                                  kind="ExternalInput")
        consts_y = nc.dram_tensor("consts_y", [128, NCC], bf16,
                                  kind="ExternalInput")
    else:
        chunks = nc.dram_tensor("chunks", [NCHUNK, 128, ccols_fused], mmdt,
                                kind="ExternalInput")
    out = nc.dram_tensor("out", [64, 512], fp32, kind="ExternalOutput")
    out_d = nc.dram_tensor("out_d", [1, 8 * len(DVE_CHUNKS)], fp32,
                           kind="ExternalOutput")

    with TileContext(nc) as tc:
        with (
            tc.tile_pool(name="consts", bufs=1) as consts,
            tc.tile_pool(name="mpool", bufs=4) as mpool,
            tc.tile_pool(name="wpool", bufs=3) as wpool,
            tc.tile_pool(name="opool", bufs=1) as opool,
            tc.tile_pool(name="psum", bufs=2, space="PSUM") as psum_pool,
        ):
            if ONCHIP_W:
                ca_sb = consts.tile([128, NCHUNK], fp32, tag="ca")
                nc.sync.dma_start(out=ca_sb[:], in_=consts_a[:])
                cy_sb = consts.tile([128, NCC], bf16, tag="cy")
                nc.gpsimd.dma_start(out=cy_sb[:, 0:WCOLS],
                                    in_=consts_y[:, 0:WCOLS])
                y_after = {0: [1], 1: [2], 4: [3], 7: [4], 10: [5]}

            acc = psum_pool.tile([64, 512], fp32)

            # PE p-state warmup: dummy matmuls during the startup window so
            # the DVFS ramp (half-speed for the first ~3us of PE activity)
            # burns on throwaway work instead of the real stream.
            warm_sb = consts.tile([128, 512], mmdt, tag="warm")
            nc.vector.memset(warm_sb[:], 1.0)
            ones_sb = consts.tile([128, 1], fp32, tag="ones")
            nc.vector.memset(ones_sb[:], 1.0)
            ndc = 8 * len(DVE_CHUNKS)
            dacc = wpool.tile([128, ndc], fp32, tag="dacc", bufs=1)
            dscr = wpool.tile([128, WCOLS], fp32, tag="dscr", bufs=1)
            dscr2 = wpool.tile([128, WCOLS], fp32, tag="dscr2", bufs=1)
            warm_acc = psum_pool.tile([64, 512], fp32, bufs=1)
            for _ in range(7):
                nc.tensor.matmul(warm_acc[:], warm_sb[:, :64], warm_sb[:],
                                 start=True, stop=True)

            n_mm = (NCHUNK - len(DVE_CHUNKS)) * NT
            mm = 0

            assert MODE in ("fp32", "fp32r", "bf16", "fp8e3") and ONCHIP_W
            # pass 1: all DMA issues (gpsimd, explicit FIFO order) and all
            # weight preps (vector, ahead of the DVE chunk work)
            mt_cur = None
            msrcs, wts = [], []
            for ch in range(NCHUNK):
                jc = ch // IPC  # chunks ordered jc-major
                if ch == 0:
                    mt = mpool.tile([128, m_cols], mmdt, tag="ms", bufs=3)
                    # split load: first matmuls gate on the first 1/3
                    nc.gpsimd.dma_start(out=mt[:, :1536],
                                        in_=m_sing[0][:, :1536])
                    nc.gpsimd.dma_start(out=mt[:, 1536:],
                                        in_=m_sing[0][:, 1536:])
                    msrc_full = mt
                elif ch >= 16:
                    mt = mpool.tile([128, m_cols], mmdt, tag="ms", bufs=3)
                    nc.gpsimd.dma_start(out=mt[:], in_=m_sing[ch - 15])
                    msrc_full = mt
                else:
                    sub = (ch - 1) % 3
                    if sub == 0:
                        mt_cur = mpool.tile([128, IPC * m_cols], mmdt,
                                            tag="mg", bufs=5)
                        nc.gpsimd.dma_start(out=mt_cur[:],
                                            in_=m_trip[(ch - 1) // 3])
                    msrc_full = mt_cur[:, sub * m_cols:(sub + 1) * m_cols]
                msrcs.append(msrc_full)
                for slc in y_after.get(ch, ()):
                    nc.gpsimd.dma_start(
                        out=cy_sb[:, slc * WCOLS:(slc + 1) * WCOLS],
                        in_=consts_y[:, slc * WCOLS:(slc + 1) * WCOLS])
                yc_src = cy_sb[:, jc * WCOLS:(jc + 1) * WCOLS]
                wt = wpool.tile([128, WCOLS], wdt, tag="w", bufs=NCHUNK)
                sc = ca_sb[:, ch:ch + 1]
                if ch == 0:
                    # split weight prep so the first matmul gates on 1/3
                    for k in range(3):
                        nc.vector.tensor_scalar_mul(
                            wt[:, k * 192:(k + 1) * 192],
                            yc_src[:, k * 192:(k + 1) * 192], sc)
                else:
                    nc.vector.tensor_scalar_mul(wt[:], yc_src, sc)
                wts.append(wt)

            # pass 2: PE matmuls, with DVE_CHUNKS offloaded to the vector
            # engine as 8 contiguous fused multiply+reduce ops per chunk
            for ch in range(NCHUNK):
                msrc_full, wt = msrcs[ch], wts[ch]
                if ch in DVE_CHUNKS:
                    di = DVE_CHUNKS.index(ch)
                    for c in range(8):
                        nc.vector.scalar_tensor_tensor(
                            out=(dscr if c % 2 == 0 else dscr2)[:],
                            in0=msrc_full[:, c * WCOLS:(c + 1) * WCOLS],
                            scalar=1.0, in1=wt[:],
                            op0=mybir.AluOpType.bypass,
                            op1=mybir.AluOpType.mult,
                            accum_out=dacc[:, di * 8 + c:di * 8 + c + 1])
                else:
                    for t in range(NT):
                        c0, c1 = 512 * t, 512 * (t + 1)
                        w0, w1 = 64 * t, 64 * (t + 1)
                        nc.tensor.matmul(
                            acc[:], wt[:, w0:w1], msrc_full[:, c0:c1],
                            start=(mm == 0), stop=(mm == n_mm - 1))
                        mm += 1
                if ch == 0:
                    # keep the PE clock hot through the triple-1 gap
                    for _ in range(4):
                        nc.tensor.matmul(warm_acc[:], warm_sb[:, :64],
                                         warm_sb[:], start=True, stop=True)

            # cross-partition sum of the DVE partials via a tiny matmul
            dout_ps = psum_pool.tile([1, ndc], fp32, tag="dout", bufs=1)
            nc.tensor.matmul(dout_ps[:], ones_sb[:, 0:1], dacc[:],
                             start=True, stop=True)
            dout_sb = opool.tile([1, ndc], fp32, tag="outd")
            nc.vector.tensor_copy(dout_sb[:], dout_ps[:])

            out_sb = opool.tile([64, 512], fp32, tag="out")
            nc.vector.tensor_copy(out_sb[:], acc[:])
            nc.sync.dma_start(out=out_d[:], in_=dout_sb[:])
            nc.sync.dma_start(out=out[:], in_=out_sb[:])

    _split_multi_waits(nc, mybir)
    return nc


def _split_multi_waits(nc, mybir):
    """This walrus build encodes at most one sync-wait per instruction
    ("Too many sync wait commands"). Tile emits up to ~2 (slot-release +
    prior-DMA WAW) and ~10 on the final drain. Hoist extra waits onto
    same-engine NoOps that execute immediately before the instruction —
    semantically identical, since sequencer waits are serial anyway."""
    skip = (mybir.InstNoOp, mybir.InstEventSemaphore,
            mybir.InstAllEngineBarrier)
    for fn in nc.m.functions:
        for blk in fn.blocks:
            idx = 0
            while idx < len(blk.instructions):
                inst = blk.instructions[idx]
                si = inst.sync_info
                if (not isinstance(inst, skip) and si is not None
                        and si.on_wait and len(si.on_wait) > 1):
                    waits = list(si.on_wait)
                    si.on_wait = [waits[-1]]
                    for w in waits[:-1]:
                        nop = mybir.InstNoOp(
                            name=nc.get_next_instruction_name(),
                            sync_info=mybir.SyncInfo(on_wait=[w],
                                                     on_update=[]),
                            engine=inst.engine,
                            bass_nofuse=True,
                        )
                        nc.register_instruction(nop)
                        blk.instructions.insert(idx, nop)
                        idx += 1
                idx += 1


def get_program():
    if "nc" not in _prog_cache:
        _prog_cache["nc"] = _build_program()
    return _prog_cache["nc"]


def _weights_and_slabs(y_rev, M, sigma, lambda_e, phi, phi_tilde):
    lam4 = lambda_e ** 0.25
    sig4 = sigma ** 0.25
    c2 = (lam4[:, None] * phi.T).astype(np.float32)        # [H, MDIM] (i,j)
    c3 = (sig4[:, None] * phi_tilde.T).astype(np.float32)  # [H, MDIM] (l,k)
    y = y_rev[:, :, 0].astype(np.float32)                  # [2m, p]

    rows = np.arange(ROWS)
    jl = rows // (H * MDIM)
    lk = rows % (H * MDIM)
    l = lk // MDIM
    kk = lk % MDIM
    jc_idx = np.arange(NJC)[:, None]
    yidx = jc_idx * JCW + jl[None, :] + kk[None, :]        # [NJC, ROWS]
    yc3 = (c3[l, kk][None, :, None] * y[yidx]).astype(np.float32)
    yc3 = yc3.reshape(NJC, 128, WCOLS)
    # partition-major [q, jc*WCOLS+col] so it loads as one DMA
    yc3 = np.ascontiguousarray(yc3.transpose(1, 0, 2).reshape(128, NCC))

    q = np.arange(128)
    c2cols = np.empty((NCORES, 128, NCHUNK), np.float32)
    for core in range(NCORES):
        for il in range(IPC):
            for jc in range(NJC):
                c2cols[core, :, jc * IPC + il] = c2[core * IPC + il,
                                                   jc * JCW + q // 16]
    return yc3, c2cols


def make_core_inputs(y_rev, M, sigma, lambda_e, phi, phi_tilde):
    """Host-side prep of the per-core device inputs for term 3."""
    yc3, c2cols = _weights_and_slabs(y_rev, M, sigma, lambda_e, phi,
                                     phi_tilde)
    yc3_j = yc3.reshape(128, NJC, WCOLS)

    in_maps = []
    for core in range(NCORES):
        slab = M[core * IPC:(core + 1) * IPC]
        slab = np.ascontiguousarray(slab).reshape(NCHUNK, 128, MCOLS)
        # reorder chunks (il, jc) -> jc-major (jc, il) to match the device
        perm = [il * NJC + jc for jc in range(NJC) for il in range(IPC)]
        slab = slab[perm]
        if MODE == "bf16x2":
            sh = _bf16(slab)
            sl = _bf16(slab - sh.astype(np.float32))
        if ONCHIP_W:
            if MODE == "bf16x2":
                mbuf = np.concatenate([sh, sl], axis=2)
            elif MODE == "bf16":
                mbuf = _bf16(slab)
            elif MODE == "fp8e3":
                import ml_dtypes
                mbuf = slab.astype(ml_dtypes.float8_e3m4)
            else:
                mbuf = slab
            mc = mbuf.shape[2]
            # DVE-offloaded chunks: (r, c, p) -> (c, r, p) so each c-slice
            # is a contiguous [128, 576] block for the vector engine
            mbuf = np.ascontiguousarray(mbuf)
            for dch in DVE_CHUNKS:
                mbuf[dch] = (mbuf[dch].reshape(128, RPP, 8, 8)
                             .transpose(0, 2, 1, 3).reshape(128, mc))
            m_trip = mbuf[1:16].reshape(5, 3, 128, mc)
            m_trip = m_trip.transpose(0, 2, 1, 3).reshape(5, 128, 3 * mc)
            in_maps.append({
                "m_sing": np.ascontiguousarray(mbuf[[0, 16, 17]]),
                "m_trip": np.ascontiguousarray(m_trip),
                "consts_a": np.ascontiguousarray(c2cols[core]),
                "consts_y": np.ascontiguousarray(_bf16(yc3)),
            })
        else:
            if MODE == "bf16x2":
                buf = np.empty((NCHUNK, 128, 2 * WCOLS + 2 * MCOLS),
                               _bf16(0.0).dtype)
                for ch in range(NCHUNK):
                    jc = ch % NJC
                    w32 = yc3_j[:, jc] * c2cols[core][:, ch:ch + 1]
                    wh = _bf16(w32)
                    wlv = _bf16(w32 - wh.astype(np.float32))
                    buf[ch, :, :WCOLS] = wh
                    buf[ch, :, WCOLS:2 * WCOLS] = wlv
                    buf[ch, :, 2 * WCOLS:2 * WCOLS + MCOLS] = sh[ch]
                    buf[ch, :, 2 * WCOLS + MCOLS:] = sl[ch]
            else:
                buf = np.empty((NCHUNK, 128, WCOLS + MCOLS), np.float32)
                for ch in range(NCHUNK):
                    jc = ch % NJC
                    buf[ch, :, :WCOLS] = yc3_j[:, jc] * \
                        c2cols[core][:, ch:ch + 1]
                    buf[ch, :, WCOLS:] = slab[ch]
            in_maps.append({"chunks": buf})
    return in_maps


def extract_term3(core_outs):
    """Gather the valid (triple, p-diagonal) entries from the per-core
    [64, 512] PSUM dumps and all-reduce over cores."""
    acc = np.zeros((64, 512), np.float64)
    for o in core_outs:
        acc += o.astype(np.float64)
    e = np.arange(8)[:, None, None]
    p = np.arange(8)[None, :, None]
    c = np.arange(8)[None, None, :]
    return acc[8 * e + p, 64 * e + 8 * c + p].sum((0, 1)).astype(np.float32)


def term3_from_results(results):
    """Full term-3: PE diagonal extraction plus the DVE-offloaded chunk
    partials ([1, 8*len(DVE_CHUNKS)] per core, columns di*8+c)."""
    t = extract_term3([r["out"] for r in results]).astype(np.float64)
    for r in results:
        t = t + r["out_d"].astype(np.float64).reshape(-1, 8).sum(0)
    return t.astype(np.float32)


def host_small_terms(y_rev, M_tilde, M, sigma, lambda_e, phi, phi_tilde):
    lam4 = lambda_e ** 0.25
    sig4 = sigma ** 0.25
    c2 = lam4[:, None] * phi.T
    c3 = sig4[:, None] * phi_tilde.T
    y_m = y_rev[:MDIM]
    u = M_tilde[0, 0] @ y_rev[0]
    u = u + np.einsum("ij,ijcp,jpq->cq", c2, M_tilde, y_m)
    u = u + np.einsum("lk,lkcp,kpq->cq", c3, M[:, :, 0, 0], y_m)
    return u.astype(np.float32)


def kernel(y_rev, M_tilde, M, sigma, lambda_e, phi, phi_tilde):
    from concourse.bass_utils import run_bass_kernel_spmd

    y_rev = np.asarray(y_rev, np.float32)
    M_tilde = np.asarray(M_tilde, np.float32)
    M = np.asarray(M, np.float32)
    sigma = np.asarray(sigma, np.float32)
    lambda_e = np.asarray(lambda_e, np.float32)
    phi = np.asarray(phi, np.float32)
    phi_tilde = np.asarray(phi_tilde, np.float32)

    nc = get_program()
    in_maps = make_core_inputs(y_rev, M, sigma, lambda_e, phi, phi_tilde)
    res = run_bass_kernel_spmd(nc, in_maps, core_ids=list(range(NCORES)))
    term3 = term3_from_results(res.results)

    u = host_small_terms(y_rev, M_tilde, M, sigma, lambda_e, phi, phi_tilde)
    return (u + term3[:, None]).astype(np.float32)



# revision 72
# speedup vs baseline: 1.1216x; 1.1216x over previous
"""Trainium2 Bass kernel for nn_DSC_28535762715377.

Computes u[c] = M_tilde[0,0] @ y_rev[0]
             + sum_ij  c2[i,j] (M_tilde[i,j] @ y_rev[j])
             + sum_lk  c3[l,k] (M[l,k,0,0] @ y_rev[k])
             + sum_ijlk c2[i,j] c3[l,k] (M[i,j,l,k] @ y_rev[j+k])

Term 3 streams the 340 MB M tensor; everything else is <1% of the bytes.
Strategy: shard M's leading i axis across 8 cores (3 i-values each,
42.5 MB contiguous per core). On each core, stream the slab through the
tensor engine as a weighted reduction: rhs tiles [128 part, 512 free]
(contiguous HBM), stationary lhsT [128, 64] holding the per-row weights
w[row,p'] = c2[i,j]*c3[l,k]*y_rev[j+k,p'] replicated over 8 p' columns
and 8 row-triples, accumulated into a single PSUM bank [64, 512].
The valid entries sit on the (triple, p'==p) diagonal; the host gathers
them and all-reduces over cores. Small terms 0-2 are computed on host.

Perf structure (fp8e3 mode, measured 53-59 us/core depending on chip
DVFS state, vs 123 us fp32r baseline):
- M quantized to fp8 e3m4 on host (1 B/elem, 10.6 MB/core): the 10.6M-
  term random sum keeps rel err at the per-element sigma (~1.3e-2 with
  bf16 weights + bf16 yc3 consts, deterministic; gate is 2e-2).
- PE streams 135 matmuls x 512 cols at 1 col/cycle @2.4 GHz = 29 us;
  DMA floor 11.1 MB at ~367 GB/s = 30 us — balanced when unthrottled.
- 3 of 18 chunks are offloaded to the otherwise-idle Vector engine as
  8 contiguous scalar_tensor_tensor fused multiply+reduce ops each
  (~760 ns per [128, 576] op; strided patterns are ~9x slower, hence
  the host-side c-major relayout of those chunks). Their [128, 1]
  partials cross-partition-sum via one ones-matmul into "out_d".
- All latency-critical DMA rides the gpsimd queue in explicit FIFO
  order (sync/scalar queues serve at only ~20-40 GB/s and steal port
  bandwidth if loaded): y0 slice, chunk0 (split so the first matmul
  gates on 1/3 of it), then chunks 1-15 as 3-wide groups whose
  partition lines are 13.8 KB (host-relaid; 4.6 KB lines only reach
  ~280 GB/s), chunks 16/17 single at the tail, remaining yc3 slices
  interleaved just ahead of their first consumer. All 5 triples stay
  resident (mg bufs=5) so the slow DVE consumption never blocks the
  DMA stream on a buffer slot.
- Dummy warmup matmuls (+4 fillers after chunk0) burn the PE DVFS ramp
  (~3 us at half clock after idle) on throwaway work.

MODE:
  "fp32"   — exact, PE-bound (fp32 matmul = 4 cycles/row).
  "fp32r"  — TF32-ish matmul (1 cycle/row), rel err ~1.5e-4.
  "bf16"   — M in bf16 (half the HBM bytes), bf16 weights, 1 pass.
             rel err ~6e-3 (harness gate is 2e-2).
  "fp8e3"  — M in fp8 e3m4 (quarter the HBM bytes), bf16 weights
             (mixed-dtype matmul), 1 pass. rel err ~1.1e-2,
             deterministic (fixed seed inputs).
  "bf16x2" — hi/lo bf16 split, 3 matmul passes (hi*hi, hi*lo, lo*hi).
             Same HBM bytes as fp32, rel err ~1e-5, PE 3 cycles/4B.
ONCHIP_W: generate weight tiles on-chip (DVE) from tiny factors instead
of streaming them fused with the M chunks (-11% HBM traffic).
"""

import numpy as np

# ---- problem constants (hardcoded; kernel.py must be self-contained) ----
H, MDIM, C, P = 24, 48, 8, 8
NCORES = 8
IPC = H // NCORES          # i-values per core = 3
NJC = 6                    # j-chunks per i
JCW = MDIM // NJC          # j per chunk = 8
ROWS = JCW * H * MDIM      # rows (of 64 floats) per chunk = 9216
RPP = ROWS // 128          # rows per partition = 72
NT = RPP // 8              # matmuls per chunk = 9
NCHUNK = IPC * NJC         # chunks per core = 18
WCOLS = RPP * 8            # 576 weight columns per chunk
MCOLS = RPP * 64           # 4608 data columns per chunk
NCC = NJC * WCOLS          # consts yc3 columns

MODE = "fp8e3"            # "fp32" | "fp32r" | "bf16" | "fp8e3" | "bf16x2"
ONCHIP_W = True
# chunks computed on the Vector engine via fused multiply+reduce instead
# of the PE, cutting the PE stream. These chunks are relaid c-major on
# host so each of the 8 c-slices is a contiguous [128, 576] block (the
# DVE is ~9x slower on strided access patterns). Their partials return
# via a tiny ones-matmul cross-partition sum in the "out_d" output.
DVE_CHUNKS = (2, 6, 10)

_prog_cache = {}


def _bf16(x):
    import ml_dtypes
    return np.asarray(x).astype(ml_dtypes.bfloat16)


def _build_program():
    import concourse.bass as bass
    import concourse.mybir as mybir
    from concourse.tile import TileContext

    fp32 = mybir.dt.float32
    bf16 = mybir.dt.bfloat16
    mmdt = {"fp32": fp32, "fp32r": mybir.dt.float32r, "bf16": bf16,
            "fp8e3": mybir.dt.float8e3, "bf16x2": bf16}[MODE]
    wdt = bf16 if MODE in ("bf16", "fp8e3") else mmdt
    nc = bass.Bass()

    # fused-chunk column layout (in mmdt elements)
    if MODE == "bf16x2":
        ccols_fused = 2 * WCOLS + 2 * MCOLS     # wh | wl | Mh | Ml
        m_cols = 2 * MCOLS                      # Mh | Ml (onchip variant)
    else:
        ccols_fused = WCOLS + MCOLS             # w | M
        m_cols = MCOLS

    fp16 = mybir.dt.float16
    if ONCHIP_W:
        # chunks jc-major, all M + yc3 traffic on the gpsimd queue in
        # explicit FIFO order (other queues are slow and steal port bw):
        # y0, c0 (split), c1, c2 singles for fast pipeline fill, then
        # triples with 13.8 KB partition lines (~367 GB/s), yc3 slices
        # interleaved just ahead of their first consumer.
        m_sing = nc.dram_tensor("m_sing", [3, 128, m_cols], mmdt,
                                kind="ExternalInput")
        m_trip = nc.dram_tensor("m_trip", [5, 128, IPC * m_cols], mmdt,
                                kind="ExternalInput")
        consts_a = nc.dram_tensor("consts_a", [128, NCHUNK], fp32,
                                  kind="ExternalInput")
        consts_y = nc.dram_tensor("consts_y", [128, NCC], bf16,
                                  kind="ExternalInput")
    else:
        chunks = nc.dram_tensor("chunks", [NCHUNK, 128, ccols_fused], mmdt,
                                kind="ExternalInput")
    out = nc.dram_tensor("out", [64, 512], fp32, kind="ExternalOutput")
    out_d = nc.dram_tensor("out_d", [1, 8 * len(DVE_CHUNKS)], fp32,
                           kind="ExternalOutput")

    with TileContext(nc) as tc:
        with (
            tc.tile_pool(name="consts", bufs=1) as consts,
            tc.tile_pool(name="mpool", bufs=4) as mpool,
            tc.tile_pool(name="wpool", bufs=3) as wpool,
            tc.tile_pool(name="opool", bufs=1) as opool,
            tc.tile_pool(name="psum", bufs=2, space="PSUM") as psum_pool,
        ):
            if ONCHIP_W:
                ca_sb = consts.tile([128, NCHUNK], fp32, tag="ca")
                nc.sync.dma_start(out=ca_sb[:], in_=consts_a[:])
                cy_sb = consts.tile([128, NCC], bf16, tag="cy")
                nc.gpsimd.dma_start(out=cy_sb[:, 0:WCOLS],
                                    in_=consts_y[:, 0:WCOLS])
                # merged slice ranges: fewer DMA_DIRECT2D issues (~680ns
                # of gpsimd engine time each) pull every later transfer in
                y_after = {0: [(1, 3)], 1: [(3, 6)]}

            acc = psum_pool.tile([64, 512], fp32)

            # PE p-state warmup: dummy matmuls during the startup window so
            # the DVFS ramp (half-speed for the first ~3us of PE activity)
            # burns on throwaway work instead of the real stream.
            warm_sb = consts.tile([128, 512], mmdt, tag="warm")
            nc.vector.memset(warm_sb[:], 1.0)
            ones_sb = consts.tile([128, 1], fp32, tag="ones")
            nc.vector.memset(ones_sb[:], 1.0)
            ndc = 8 * len(DVE_CHUNKS)
            dacc = wpool.tile([128, ndc], fp32, tag="dacc", bufs=1)
            dscr = wpool.tile([128, WCOLS], fp32, tag="dscr", bufs=1)
            dscr2 = wpool.tile([128, WCOLS], fp32, tag="dscr2", bufs=1)
            warm_acc = psum_pool.tile([64, 512], fp32, bufs=1)
            for _ in range(7):
                nc.tensor.matmul(warm_acc[:], warm_sb[:, :64], warm_sb[:],
                                 start=True, stop=True)

            n_mm = (NCHUNK - len(DVE_CHUNKS)) * NT
            mm = 0

            assert MODE in ("fp32", "fp32r", "bf16", "fp8e3") and ONCHIP_W
            # pass 1: all DMA issues (gpsimd, explicit FIFO order) and all
            # weight preps (vector, ahead of the DVE chunk work)
            mt_cur = None
            msrcs, wts = [], []
            for ch in range(NCHUNK):
                jc = ch // IPC  # chunks ordered jc-major
                if ch == 0:
                    mt = mpool.tile([128, m_cols], mmdt, tag="ms", bufs=3)
                    # split load: first matmuls gate on the first 1/3
                    nc.gpsimd.dma_start(out=mt[:, :1536],
                                        in_=m_sing[0][:, :1536])
                    nc.gpsimd.dma_start(out=mt[:, 1536:],
                                        in_=m_sing[0][:, 1536:])
                    msrc_full = mt
                elif ch >= 16:
                    mt = mpool.tile([128, m_cols], mmdt, tag="ms", bufs=3)
                    nc.gpsimd.dma_start(out=mt[:], in_=m_sing[ch - 15])
                    msrc_full = mt
                else:
                    sub = (ch - 1) % 3
                    if sub == 0:
                        mt_cur = mpool.tile([128, IPC * m_cols], mmdt,
                                            tag="mg", bufs=5)
                        nc.gpsimd.dma_start(out=mt_cur[:],
                                            in_=m_trip[(ch - 1) // 3])
                    msrc_full = mt_cur[:, sub * m_cols:(sub + 1) * m_cols]
                msrcs.append(msrc_full)
                for s0, s1 in y_after.get(ch, ()):
                    nc.gpsimd.dma_start(
                        out=cy_sb[:, s0 * WCOLS:s1 * WCOLS],
                        in_=consts_y[:, s0 * WCOLS:s1 * WCOLS])
                yc_src = cy_sb[:, jc * WCOLS:(jc + 1) * WCOLS]
                wt = wpool.tile([128, WCOLS], wdt, tag="w", bufs=NCHUNK)
                sc = ca_sb[:, ch:ch + 1]
                if ch == 0:
                    # split weight prep so the first matmul gates on 1/3
                    for k in range(3):
                        nc.vector.tensor_scalar_mul(
                            wt[:, k * 192:(k + 1) * 192],
                            yc_src[:, k * 192:(k + 1) * 192], sc)
                else:
                    nc.vector.tensor_scalar_mul(wt[:], yc_src, sc)
                wts.append(wt)

            # pass 2: PE matmuls, with DVE_CHUNKS offloaded to the vector
            # engine as 8 contiguous fused multiply+reduce ops per chunk
            for ch in range(NCHUNK):
                msrc_full, wt = msrcs[ch], wts[ch]
                if ch in DVE_CHUNKS:
                    di = DVE_CHUNKS.index(ch)
                    for c in range(8):
                        nc.vector.scalar_tensor_tensor(
                            out=(dscr if c % 2 == 0 else dscr2)[:],
                            in0=msrc_full[:, c * WCOLS:(c + 1) * WCOLS],
                            scalar=1.0, in1=wt[:],
                            op0=mybir.AluOpType.bypass,
                            op1=mybir.AluOpType.mult,
                            accum_out=dacc[:, di * 8 + c:di * 8 + c + 1])
                else:
                    for t in range(NT):
                        c0, c1 = 512 * t, 512 * (t + 1)
                        w0, w1 = 64 * t, 64 * (t + 1)
                        nc.tensor.matmul(
                            acc[:], wt[:, w0:w1], msrc_full[:, c0:c1],
                            start=(mm == 0), stop=(mm == n_mm - 1))
                        mm += 1
                if ch == 0:
                    # keep the PE clock hot through the triple-1 gap
                    for _ in range(4):
                        nc.tensor.matmul(warm_acc[:], warm_sb[:, :64],
                                         warm_sb[:], start=True, stop=True)

            # cross-partition sum of the DVE partials via a tiny matmul
            dout_ps = psum_pool.tile([1, ndc], fp32, tag="dout", bufs=1)
            nc.tensor.matmul(dout_ps[:], ones_sb[:, 0:1], dacc[:],
                             start=True, stop=True)
            dout_sb = opool.tile([1, ndc], fp32, tag="outd")
            nc.vector.tensor_copy(dout_sb[:], dout_ps[:])

            out_sb = opool.tile([64, 512], fp32, tag="out")
            nc.vector.tensor_copy(out_sb[:], acc[:])
            nc.sync.dma_start(out=out_d[:], in_=dout_sb[:])
            nc.sync.dma_start(out=out[:], in_=out_sb[:])

    _split_multi_waits(nc, mybir)
    return nc


def _split_multi_waits(nc, mybir):
    """This walrus build encodes at most one sync-wait per instruction
    ("Too many sync wait commands"). Tile emits up to ~2 (slot-release +
    prior-DMA WAW) and ~10 on the final drain. Hoist extra waits onto
    same-engine NoOps that execute immediately before the instruction —
    semantically identical, since sequencer waits are serial anyway."""
    skip = (mybir.InstNoOp, mybir.InstEventSemaphore,
            mybir.InstAllEngineBarrier)
    for fn in nc.m.functions:
        for blk in fn.blocks:
            idx = 0
            while idx < len(blk.instructions):
                inst = blk.instructions[idx]
                si = inst.sync_info
                if (not isinstance(inst, skip) and si is not None
                        and si.on_wait and len(si.on_wait) > 1):
                    waits = list(si.on_wait)
                    si.on_wait = [waits[-1]]
                    for w in waits[:-1]:
                        nop = mybir.InstNoOp(
                            name=nc.get_next_instruction_name(),
                            sync_info=mybir.SyncInfo(on_wait=[w],
                                                     on_update=[]),
                            engine=inst.engine,
                            bass_nofuse=True,
                        )
                        nc.register_instruction(nop)
                        blk.instructions.insert(idx, nop)
                        idx += 1
                idx += 1


def get_program():
    if "nc" not in _prog_cache:
        _prog_cache["nc"] = _build_program()
    return _prog_cache["nc"]


def _weights_and_slabs(y_rev, M, sigma, lambda_e, phi, phi_tilde):
    lam4 = lambda_e ** 0.25
    sig4 = sigma ** 0.25
    c2 = (lam4[:, None] * phi.T).astype(np.float32)        # [H, MDIM] (i,j)
    c3 = (sig4[:, None] * phi_tilde.T).astype(np.float32)  # [H, MDIM] (l,k)
    y = y_rev[:, :, 0].astype(np.float32)                  # [2m, p]

    rows = np.arange(ROWS)
    jl = rows // (H * MDIM)
    lk = rows % (H * MDIM)
    l = lk // MDIM
    kk = lk % MDIM
    jc_idx = np.arange(NJC)[:, None]
    yidx = jc_idx * JCW + jl[None, :] + kk[None, :]        # [NJC, ROWS]
    yc3 = (c3[l, kk][None, :, None] * y[yidx]).astype(np.float32)
    yc3 = yc3.reshape(NJC, 128, WCOLS)
    # partition-major [q, jc*WCOLS+col] so it loads as one DMA
    yc3 = np.ascontiguousarray(yc3.transpose(1, 0, 2).reshape(128, NCC))

    q = np.arange(128)
    c2cols = np.empty((NCORES, 128, NCHUNK), np.float32)
    for core in range(NCORES):
        for il in range(IPC):
            for jc in range(NJC):
                c2cols[core, :, jc * IPC + il] = c2[core * IPC + il,
                                                   jc * JCW + q // 16]
    return yc3, c2cols


def make_core_inputs(y_rev, M, sigma, lambda_e, phi, phi_tilde):
    """Host-side prep of the per-core device inputs for term 3."""
    yc3, c2cols = _weights_and_slabs(y_rev, M, sigma, lambda_e, phi,
                                     phi_tilde)
    yc3_j = yc3.reshape(128, NJC, WCOLS)

    in_maps = []
    for core in range(NCORES):
        slab = M[core * IPC:(core + 1) * IPC]
        slab = np.ascontiguousarray(slab).reshape(NCHUNK, 128, MCOLS)
        # reorder chunks (il, jc) -> jc-major (jc, il) to match the device
        perm = [il * NJC + jc for jc in range(NJC) for il in range(IPC)]
        slab = slab[perm]
        if MODE == "bf16x2":
            sh = _bf16(slab)
            sl = _bf16(slab - sh.astype(np.float32))
        if ONCHIP_W:
            if MODE == "bf16x2":
                mbuf = np.concatenate([sh, sl], axis=2)
            elif MODE == "bf16":
                mbuf = _bf16(slab)
            elif MODE == "fp8e3":
                import ml_dtypes
                mbuf = slab.astype(ml_dtypes.float8_e3m4)
            else:
                mbuf = slab
            mc = mbuf.shape[2]
            # DVE-offloaded chunks: (r, c, p) -> (c, r, p) so each c-slice
            # is a contiguous [128, 576] block for the vector engine
            mbuf = np.ascontiguousarray(mbuf)
            for dch in DVE_CHUNKS:
                mbuf[dch] = (mbuf[dch].reshape(128, RPP, 8, 8)
                             .transpose(0, 2, 1, 3).reshape(128, mc))
            m_trip = mbuf[1:16].reshape(5, 3, 128, mc)
            m_trip = m_trip.transpose(0, 2, 1, 3).reshape(5, 128, 3 * mc)
            in_maps.append({
                "m_sing": np.ascontiguousarray(mbuf[[0, 16, 17]]),
                "m_trip": np.ascontiguousarray(m_trip),
                "consts_a": np.ascontiguousarray(c2cols[core]),
                "consts_y": np.ascontiguousarray(_bf16(yc3)),
            })
        else:
            if MODE == "bf16x2":
                buf = np.empty((NCHUNK, 128, 2 * WCOLS + 2 * MCOLS),
                               _bf16(0.0).dtype)
                for ch in range(NCHUNK):
                    jc = ch % NJC
                    w32 = yc3_j[:, jc] * c2cols[core][:, ch:ch + 1]
                    wh = _bf16(w32)
                    wlv = _bf16(w32 - wh.astype(np.float32))
                    buf[ch, :, :WCOLS] = wh
                    buf[ch, :, WCOLS:2 * WCOLS] = wlv
                    buf[ch, :, 2 * WCOLS:2 * WCOLS + MCOLS] = sh[ch]
                    buf[ch, :, 2 * WCOLS + MCOLS:] = sl[ch]
            else:
                buf = np.empty((NCHUNK, 128, WCOLS + MCOLS), np.float32)
                for ch in range(NCHUNK):
                    jc = ch % NJC
                    buf[ch, :, :WCOLS] = yc3_j[:, jc] * \
                        c2cols[core][:, ch:ch + 1]
                    buf[ch, :, WCOLS:] = slab[ch]
            in_maps.append({"chunks": buf})
    return in_maps


def extract_term3(core_outs):
    """Gather the valid (triple, p-diagonal) entries from the per-core
    [64, 512] PSUM dumps and all-reduce over cores."""
    acc = np.zeros((64, 512), np.float64)
    for o in core_outs:
        acc += o.astype(np.float64)
    e = np.arange(8)[:, None, None]
    p = np.arange(8)[None, :, None]
    c = np.arange(8)[None, None, :]
    return acc[8 * e + p, 64 * e + 8 * c + p].sum((0, 1)).astype(np.float32)


def term3_from_results(results):
    """Full term-3: PE diagonal extraction plus the DVE-offloaded chunk
    partials ([1, 8*len(DVE_CHUNKS)] per core, columns di*8+c)."""
    t = extract_term3([r["out"] for r in results]).astype(np.float64)
    for r in results:
        t = t + r["out_d"].astype(np.float64).reshape(-1, 8).sum(0)
    return t.astype(np.float32)


def host_small_terms(y_rev, M_tilde, M, sigma, lambda_e, phi, phi_tilde):
    lam4 = lambda_e ** 0.25
    sig4 = sigma ** 0.25
    c2 = lam4[:, None] * phi.T
    c3 = sig4[:, None] * phi_tilde.T
    y_m = y_rev[:MDIM]
    u = M_tilde[0, 0] @ y_rev[0]
    u = u + np.einsum("ij,ijcp,jpq->cq", c2, M_tilde, y_m)
    u = u + np.einsum("lk,lkcp,kpq->cq", c3, M[:, :, 0, 0], y_m)
    return u.astype(np.float32)


def kernel(y_rev, M_tilde, M, sigma, lambda_e, phi, phi_tilde):
    from concourse.bass_utils import run_bass_kernel_spmd

    y_rev = np.asarray(y_rev, np.float32)
    M_tilde = np.asarray(M_tilde, np.float32)
    M = np.asarray(M, np.float32)
    sigma = np.asarray(sigma, np.float32)
    lambda_e = np.asarray(lambda_e, np.float32)
    phi = np.asarray(phi, np.float32)
    phi_tilde = np.asarray(phi_tilde, np.float32)

    nc = get_program()
    in_maps = make_core_inputs(y_rev, M, sigma, lambda_e, phi, phi_tilde)
    res = run_bass_kernel_spmd(nc, in_maps, core_ids=list(range(NCORES)))
    term3 = term3_from_results(res.results)

    u = host_small_terms(y_rev, M_tilde, M, sigma, lambda_e, phi, phi_tilde)
    return (u + term3[:, None]).astype(np.float32)



# revision 73
# speedup vs baseline: 1.1747x; 1.0473x over previous
"""Trainium2 Bass kernel for nn_DSC_28535762715377.

Computes u[c] = M_tilde[0,0] @ y_rev[0]
             + sum_ij  c2[i,j] (M_tilde[i,j] @ y_rev[j])
             + sum_lk  c3[l,k] (M[l,k,0,0] @ y_rev[k])
             + sum_ijlk c2[i,j] c3[l,k] (M[i,j,l,k] @ y_rev[j+k])

Term 3 streams the 340 MB M tensor; everything else is <1% of the bytes.
Strategy: shard M's leading i axis across 8 cores (3 i-values each,
42.5 MB contiguous per core). On each core, stream the slab through the
tensor engine as a weighted reduction: rhs tiles [128 part, 512 free]
(contiguous HBM), stationary lhsT [128, 64] holding the per-row weights
w[row,p'] = c2[i,j]*c3[l,k]*y_rev[j+k,p'] replicated over 8 p' columns
and 8 row-triples, accumulated into a single PSUM bank [64, 512].
The valid entries sit on the (triple, p'==p) diagonal; the host gathers
them and all-reduces over cores. Small terms 0-2 are computed on host.

Perf structure (fp8e3 mode, measured 53-59 us/core depending on chip
DVFS state, vs 123 us fp32r baseline):
- M quantized to fp8 e3m4 on host (1 B/elem, 10.6 MB/core): the 10.6M-
  term random sum keeps rel err at the per-element sigma (~1.3e-2 with
  bf16 weights + bf16 yc3 consts, deterministic; gate is 2e-2).
- PE streams 135 matmuls x 512 cols at 1 col/cycle @2.4 GHz = 29 us;
  DMA floor 11.1 MB at ~367 GB/s = 30 us — balanced when unthrottled.
- 3 of 18 chunks are offloaded to the otherwise-idle Vector engine as
  8 contiguous scalar_tensor_tensor fused multiply+reduce ops each
  (~760 ns per [128, 576] op; strided patterns are ~9x slower, hence
  the host-side c-major relayout of those chunks). Their [128, 1]
  partials cross-partition-sum via one ones-matmul into "out_d".
- All latency-critical DMA rides the gpsimd queue in explicit FIFO
  order (sync/scalar queues serve at only ~20-40 GB/s and steal port
  bandwidth if loaded): y0 slice, chunk0 (split so the first matmul
  gates on 1/3 of it), then chunks 1-15 as 3-wide groups whose
  partition lines are 13.8 KB (host-relaid; 4.6 KB lines only reach
  ~280 GB/s), chunks 16/17 single at the tail, remaining yc3 slices
  interleaved just ahead of their first consumer. All 5 triples stay
  resident (mg bufs=5) so the slow DVE consumption never blocks the
  DMA stream on a buffer slot.
- Dummy warmup matmuls (+4 fillers after chunk0) burn the PE DVFS ramp
  (~3 us at half clock after idle) on throwaway work.

MODE:
  "fp32"   — exact, PE-bound (fp32 matmul = 4 cycles/row).
  "fp32r"  — TF32-ish matmul (1 cycle/row), rel err ~1.5e-4.
  "bf16"   — M in bf16 (half the HBM bytes), bf16 weights, 1 pass.
             rel err ~6e-3 (harness gate is 2e-2).
  "fp8e3"  — M in fp8 e3m4 (quarter the HBM bytes), bf16 weights
             (mixed-dtype matmul), 1 pass. rel err ~1.1e-2,
             deterministic (fixed seed inputs).
  "bf16x2" — hi/lo bf16 split, 3 matmul passes (hi*hi, hi*lo, lo*hi).
             Same HBM bytes as fp32, rel err ~1e-5, PE 3 cycles/4B.
ONCHIP_W: generate weight tiles on-chip (DVE) from tiny factors instead
of streaming them fused with the M chunks (-11% HBM traffic).
"""

import numpy as np

# ---- problem constants (hardcoded; kernel.py must be self-contained) ----
H, MDIM, C, P = 24, 48, 8, 8
NCORES = 8
IPC = H // NCORES          # i-values per core = 3
NJC = 6                    # j-chunks per i
JCW = MDIM // NJC          # j per chunk = 8
ROWS = JCW * H * MDIM      # rows (of 64 floats) per chunk = 9216
RPP = ROWS // 128          # rows per partition = 72
NT = RPP // 8              # matmuls per chunk = 9
NCHUNK = IPC * NJC         # chunks per core = 18
WCOLS = RPP * 8            # 576 weight columns per chunk
MCOLS = RPP * 64           # 4608 data columns per chunk
NCC = NJC * WCOLS          # consts yc3 columns

MODE = "fp8e3"            # "fp32" | "fp32r" | "bf16" | "fp8e3" | "bf16x2"
ONCHIP_W = True
# chunks computed on the Vector engine via fused multiply+reduce instead
# of the PE, cutting the PE stream. These chunks are relaid c-major on
# host so each of the 8 c-slices is a contiguous [128, 576] block (the
# DVE is ~9x slower on strided access patterns). Their partials return
# via a tiny ones-matmul cross-partition sum in the "out_d" output.
DVE_CHUNKS = (2, 6, 10, 14)

_prog_cache = {}


def _bf16(x):
    import ml_dtypes
    return np.asarray(x).astype(ml_dtypes.bfloat16)


def _build_program():
    import concourse.bass as bass
    import concourse.mybir as mybir
    from concourse.tile import TileContext

    fp32 = mybir.dt.float32
    bf16 = mybir.dt.bfloat16
    mmdt = {"fp32": fp32, "fp32r": mybir.dt.float32r, "bf16": bf16,
            "fp8e3": mybir.dt.float8e3, "bf16x2": bf16}[MODE]
    wdt = bf16 if MODE in ("bf16", "fp8e3") else mmdt
    nc = bass.Bass()

    # fused-chunk column layout (in mmdt elements)
    if MODE == "bf16x2":
        ccols_fused = 2 * WCOLS + 2 * MCOLS     # wh | wl | Mh | Ml
        m_cols = 2 * MCOLS                      # Mh | Ml (onchip variant)
    else:
        ccols_fused = WCOLS + MCOLS             # w | M
        m_cols = MCOLS

    fp16 = mybir.dt.float16
    if ONCHIP_W:
        # chunks jc-major, all M + yc3 traffic on the gpsimd queue in
        # explicit FIFO order (other queues are slow and steal port bw):
        # y0, c0 (split), c1, c2 singles for fast pipeline fill, then
        # triples with 13.8 KB partition lines (~367 GB/s), yc3 slices
        # interleaved just ahead of their first consumer.
        m_sing = nc.dram_tensor("m_sing", [3, 128, m_cols], mmdt,
                                kind="ExternalInput")
        m_trip = nc.dram_tensor("m_trip", [5, 128, IPC * m_cols], mmdt,
                                kind="ExternalInput")
        consts_a = nc.dram_tensor("consts_a", [128, NCHUNK], fp32,
                                  kind="ExternalInput")
        consts_y = nc.dram_tensor("consts_y", [128, NCC], bf16,
                                  kind="ExternalInput")
    else:
        chunks = nc.dram_tensor("chunks", [NCHUNK, 128, ccols_fused], mmdt,
                                kind="ExternalInput")
    out = nc.dram_tensor("out", [64, 512], fp32, kind="ExternalOutput")
    out_d = nc.dram_tensor("out_d", [1, 8 * len(DVE_CHUNKS)], fp32,
                           kind="ExternalOutput")

    with TileContext(nc) as tc:
        with (
            tc.tile_pool(name="consts", bufs=1) as consts,
            tc.tile_pool(name="mpool", bufs=4) as mpool,
            tc.tile_pool(name="wpool", bufs=3) as wpool,
            tc.tile_pool(name="opool", bufs=1) as opool,
            tc.tile_pool(name="psum", bufs=2, space="PSUM") as psum_pool,
        ):
            if ONCHIP_W:
                ca_sb = consts.tile([128, NCHUNK], fp32, tag="ca")
                nc.sync.dma_start(out=ca_sb[:], in_=consts_a[:])
                cy_sb = consts.tile([128, NCC], bf16, tag="cy")
                nc.gpsimd.dma_start(out=cy_sb[:, 0:WCOLS],
                                    in_=consts_y[:, 0:WCOLS])
                # merged slice ranges: fewer DMA_DIRECT2D issues (~680ns
                # of gpsimd engine time each) pull every later transfer in
                y_after = {0: [(1, 3)], 1: [(3, 6)]}

            acc = psum_pool.tile([64, 512], fp32)

            # PE p-state warmup: dummy matmuls during the startup window so
            # the DVFS ramp (half-speed for the first ~3us of PE activity)
            # burns on throwaway work instead of the real stream.
            warm_sb = consts.tile([128, 512], mmdt, tag="warm")
            nc.vector.memset(warm_sb[:], 1.0)
            ones_sb = consts.tile([128, 1], fp32, tag="ones")
            nc.vector.memset(ones_sb[:], 1.0)
            ndc = 8 * len(DVE_CHUNKS)
            dacc = wpool.tile([128, ndc], fp32, tag="dacc", bufs=1)
            dscr = wpool.tile([128, WCOLS], fp32, tag="dscr", bufs=1)
            dscr2 = wpool.tile([128, WCOLS], fp32, tag="dscr2", bufs=1)
            warm_acc = psum_pool.tile([64, 512], fp32, bufs=1)
            for _ in range(7):
                nc.tensor.matmul(warm_acc[:], warm_sb[:, :64], warm_sb[:],
                                 start=True, stop=True)

            n_mm = (NCHUNK - len(DVE_CHUNKS)) * NT
            mm = 0

            assert MODE in ("fp32", "fp32r", "bf16", "fp8e3") and ONCHIP_W
            # pass 1: all DMA issues (gpsimd, explicit FIFO order) and all
            # weight preps (vector, ahead of the DVE chunk work)
            mt_cur = None
            msrcs, wts = [], []
            for ch in range(NCHUNK):
                jc = ch // IPC  # chunks ordered jc-major
                if ch == 0:
                    mt = mpool.tile([128, m_cols], mmdt, tag="ms", bufs=3)
                    # split load: first matmuls gate on the first 1/3
                    nc.gpsimd.dma_start(out=mt[:, :1536],
                                        in_=m_sing[0][:, :1536])
                    nc.gpsimd.dma_start(out=mt[:, 1536:],
                                        in_=m_sing[0][:, 1536:])
                    msrc_full = mt
                elif ch >= 16:
                    mt = mpool.tile([128, m_cols], mmdt, tag="ms", bufs=3)
                    nc.gpsimd.dma_start(out=mt[:], in_=m_sing[ch - 15])
                    msrc_full = mt
                else:
                    sub = (ch - 1) % 3
                    if sub == 0:
                        mt_cur = mpool.tile([128, IPC * m_cols], mmdt,
                                            tag="mg", bufs=5)
                        nc.gpsimd.dma_start(out=mt_cur[:],
                                            in_=m_trip[(ch - 1) // 3])
                    msrc_full = mt_cur[:, sub * m_cols:(sub + 1) * m_cols]
                msrcs.append(msrc_full)
                for s0, s1 in y_after.get(ch, ()):
                    nc.gpsimd.dma_start(
                        out=cy_sb[:, s0 * WCOLS:s1 * WCOLS],
                        in_=consts_y[:, s0 * WCOLS:s1 * WCOLS])
                yc_src = cy_sb[:, jc * WCOLS:(jc + 1) * WCOLS]
                wt = wpool.tile([128, WCOLS], wdt, tag="w", bufs=NCHUNK)
                sc = ca_sb[:, ch:ch + 1]
                if ch == 0:
                    # split weight prep so the first matmul gates on 1/3
                    for k in range(3):
                        nc.vector.tensor_scalar_mul(
                            wt[:, k * 192:(k + 1) * 192],
                            yc_src[:, k * 192:(k + 1) * 192], sc)
                else:
                    nc.vector.tensor_scalar_mul(wt[:], yc_src, sc)
                wts.append(wt)

            # pass 2: PE matmuls, with DVE_CHUNKS offloaded to the vector
            # engine as 8 contiguous fused multiply+reduce ops per chunk
            for ch in range(NCHUNK):
                msrc_full, wt = msrcs[ch], wts[ch]
                if ch in DVE_CHUNKS:
                    di = DVE_CHUNKS.index(ch)
                    for c in range(8):
                        nc.vector.scalar_tensor_tensor(
                            out=(dscr if c % 2 == 0 else dscr2)[:],
                            in0=msrc_full[:, c * WCOLS:(c + 1) * WCOLS],
                            scalar=1.0, in1=wt[:],
                            op0=mybir.AluOpType.bypass,
                            op1=mybir.AluOpType.mult,
                            accum_out=dacc[:, di * 8 + c:di * 8 + c + 1])
                else:
                    for t in range(NT):
                        c0, c1 = 512 * t, 512 * (t + 1)
                        w0, w1 = 64 * t, 64 * (t + 1)
                        nc.tensor.matmul(
                            acc[:], wt[:, w0:w1], msrc_full[:, c0:c1],
                            start=(mm == 0), stop=(mm == n_mm - 1))
                        mm += 1
                if ch == 0:
                    # keep the PE clock hot through the triple-1 gap
                    for _ in range(4):
                        nc.tensor.matmul(warm_acc[:], warm_sb[:, :64],
                                         warm_sb[:], start=True, stop=True)

            # cross-partition sum of the DVE partials via a tiny matmul
            dout_ps = psum_pool.tile([1, ndc], fp32, tag="dout", bufs=1)
            nc.tensor.matmul(dout_ps[:], ones_sb[:, 0:1], dacc[:],
                             start=True, stop=True)
            dout_sb = opool.tile([1, ndc], fp32, tag="outd")
            nc.vector.tensor_copy(dout_sb[:], dout_ps[:])

            out_sb = opool.tile([64, 512], fp32, tag="out")
            nc.vector.tensor_copy(out_sb[:], acc[:])
            nc.sync.dma_start(out=out_d[:], in_=dout_sb[:])
            nc.sync.dma_start(out=out[:], in_=out_sb[:])

    _split_multi_waits(nc, mybir)
    return nc


def _split_multi_waits(nc, mybir):
    """This walrus build encodes at most one sync-wait per instruction
    ("Too many sync wait commands"). Tile emits up to ~2 (slot-release +
    prior-DMA WAW) and ~10 on the final drain. Hoist extra waits onto
    same-engine NoOps that execute immediately before the instruction —
    semantically identical, since sequencer waits are serial anyway."""
    skip = (mybir.InstNoOp, mybir.InstEventSemaphore,
            mybir.InstAllEngineBarrier)
    for fn in nc.m.functions:
        for blk in fn.blocks:
            idx = 0
            while idx < len(blk.instructions):
                inst = blk.instructions[idx]
                si = inst.sync_info
                if (not isinstance(inst, skip) and si is not None
                        and si.on_wait and len(si.on_wait) > 1):
                    waits = list(si.on_wait)
                    si.on_wait = [waits[-1]]
                    for w in waits[:-1]:
                        nop = mybir.InstNoOp(
                            name=nc.get_next_instruction_name(),
                            sync_info=mybir.SyncInfo(on_wait=[w],
                                                     on_update=[]),
                            engine=inst.engine,
                            bass_nofuse=True,
                        )
                        nc.register_instruction(nop)
                        blk.instructions.insert(idx, nop)
                        idx += 1
                idx += 1


def get_program():
    if "nc" not in _prog_cache:
        _prog_cache["nc"] = _build_program()
    return _prog_cache["nc"]


def _weights_and_slabs(y_rev, M, sigma, lambda_e, phi, phi_tilde):
    lam4 = lambda_e ** 0.25
    sig4 = sigma ** 0.25
    c2 = (lam4[:, None] * phi.T).astype(np.float32)        # [H, MDIM] (i,j)
    c3 = (sig4[:, None] * phi_tilde.T).astype(np.float32)  # [H, MDIM] (l,k)
    y = y_rev[:, :, 0].astype(np.float32)                  # [2m, p]

    rows = np.arange(ROWS)
    jl = rows // (H * MDIM)
    lk = rows % (H * MDIM)
    l = lk // MDIM
    kk = lk % MDIM
    jc_idx = np.arange(NJC)[:, None]
    yidx = jc_idx * JCW + jl[None, :] + kk[None, :]        # [NJC, ROWS]
    yc3 = (c3[l, kk][None, :, None] * y[yidx]).astype(np.float32)
    yc3 = yc3.reshape(NJC, 128, WCOLS)
    # partition-major [q, jc*WCOLS+col] so it loads as one DMA
    yc3 = np.ascontiguousarray(yc3.transpose(1, 0, 2).reshape(128, NCC))

    q = np.arange(128)
    c2cols = np.empty((NCORES, 128, NCHUNK), np.float32)
    for core in range(NCORES):
        for il in range(IPC):
            for jc in range(NJC):
                c2cols[core, :, jc * IPC + il] = c2[core * IPC + il,
                                                   jc * JCW + q // 16]
    return yc3, c2cols


def make_core_inputs(y_rev, M, sigma, lambda_e, phi, phi_tilde):
    """Host-side prep of the per-core device inputs for term 3."""
    yc3, c2cols = _weights_and_slabs(y_rev, M, sigma, lambda_e, phi,
                                     phi_tilde)
    yc3_j = yc3.reshape(128, NJC, WCOLS)

    in_maps = []
    for core in range(NCORES):
        slab = M[core * IPC:(core + 1) * IPC]
        slab = np.ascontiguousarray(slab).reshape(NCHUNK, 128, MCOLS)
        # reorder chunks (il, jc) -> jc-major (jc, il) to match the device
        perm = [il * NJC + jc for jc in range(NJC) for il in range(IPC)]
        slab = slab[perm]
        if MODE == "bf16x2":
            sh = _bf16(slab)
            sl = _bf16(slab - sh.astype(np.float32))
        if ONCHIP_W:
            if MODE == "bf16x2":
                mbuf = np.concatenate([sh, sl], axis=2)
            elif MODE == "bf16":
                mbuf = _bf16(slab)
            elif MODE == "fp8e3":
                import ml_dtypes
                mbuf = slab.astype(ml_dtypes.float8_e3m4)
            else:
                mbuf = slab
            mc = mbuf.shape[2]
            # DVE-offloaded chunks: (r, c, p) -> (c, r, p) so each c-slice
            # is a contiguous [128, 576] block for the vector engine
            mbuf = np.ascontiguousarray(mbuf)
            for dch in DVE_CHUNKS:
                mbuf[dch] = (mbuf[dch].reshape(128, RPP, 8, 8)
                             .transpose(0, 2, 1, 3).reshape(128, mc))
            m_trip = mbuf[1:16].reshape(5, 3, 128, mc)
            m_trip = m_trip.transpose(0, 2, 1, 3).reshape(5, 128, 3 * mc)
            in_maps.append({
                "m_sing": np.ascontiguousarray(mbuf[[0, 16, 17]]),
                "m_trip": np.ascontiguousarray(m_trip),
                "consts_a": np.ascontiguousarray(c2cols[core]),
                "consts_y": np.ascontiguousarray(_bf16(yc3)),
            })
        else:
            if MODE == "bf16x2":
                buf = np.empty((NCHUNK, 128, 2 * WCOLS + 2 * MCOLS),
                               _bf16(0.0).dtype)
                for ch in range(NCHUNK):
                    jc = ch % NJC
                    w32 = yc3_j[:, jc] * c2cols[core][:, ch:ch + 1]
                    wh = _bf16(w32)
                    wlv = _bf16(w32 - wh.astype(np.float32))
                    buf[ch, :, :WCOLS] = wh
                    buf[ch, :, WCOLS:2 * WCOLS] = wlv
                    buf[ch, :, 2 * WCOLS:2 * WCOLS + MCOLS] = sh[ch]
                    buf[ch, :, 2 * WCOLS + MCOLS:] = sl[ch]
            else:
                buf = np.empty((NCHUNK, 128, WCOLS + MCOLS), np.float32)
                for ch in range(NCHUNK):
                    jc = ch % NJC
                    buf[ch, :, :WCOLS] = yc3_j[:, jc] * \
                        c2cols[core][:, ch:ch + 1]
                    buf[ch, :, WCOLS:] = slab[ch]
            in_maps.append({"chunks": buf})
    return in_maps


def extract_term3(core_outs):
    """Gather the valid (triple, p-diagonal) entries from the per-core
    [64, 512] PSUM dumps and all-reduce over cores."""
    acc = np.zeros((64, 512), np.float64)
    for o in core_outs:
        acc += o.astype(np.float64)
    e = np.arange(8)[:, None, None]
    p = np.arange(8)[None, :, None]
    c = np.arange(8)[None, None, :]
    return acc[8 * e + p, 64 * e + 8 * c + p].sum((0, 1)).astype(np.float32)


def term3_from_results(results):
    """Full term-3: PE diagonal extraction plus the DVE-offloaded chunk
    partials ([1, 8*len(DVE_CHUNKS)] per core, columns di*8+c)."""
    t = extract_term3([r["out"] for r in results]).astype(np.float64)
    for r in results:
        t = t + r["out_d"].astype(np.float64).reshape(-1, 8).sum(0)
    return t.astype(np.float32)


def host_small_terms(y_rev, M_tilde, M, sigma, lambda_e, phi, phi_tilde):
    lam4 = lambda_e ** 0.25
    sig4 = sigma ** 0.25
    c2 = lam4[:, None] * phi.T
    c3 = sig4[:, None] * phi_tilde.T
    y_m = y_rev[:MDIM]
    u = M_tilde[0, 0] @ y_rev[0]
    u = u + np.einsum("ij,ijcp,jpq->cq", c2, M_tilde, y_m)
    u = u + np.einsum("lk,lkcp,kpq->cq", c3, M[:, :, 0, 0], y_m)
    return u.astype(np.float32)


def kernel(y_rev, M_tilde, M, sigma, lambda_e, phi, phi_tilde):
    from concourse.bass_utils import run_bass_kernel_spmd

    y_rev = np.asarray(y_rev, np.float32)
    M_tilde = np.asarray(M_tilde, np.float32)
    M = np.asarray(M, np.float32)
    sigma = np.asarray(sigma, np.float32)
    lambda_e = np.asarray(lambda_e, np.float32)
    phi = np.asarray(phi, np.float32)
    phi_tilde = np.asarray(phi_tilde, np.float32)

    nc = get_program()
    in_maps = make_core_inputs(y_rev, M, sigma, lambda_e, phi, phi_tilde)
    res = run_bass_kernel_spmd(nc, in_maps, core_ids=list(range(NCORES)))
    term3 = term3_from_results(res.results)

    u = host_small_terms(y_rev, M_tilde, M, sigma, lambda_e, phi, phi_tilde)
    return (u + term3[:, None]).astype(np.float32)



# revision 74
# speedup vs baseline: 1.2136x; 1.0331x over previous
"""Trainium2 Bass kernel for nn_DSC_28535762715377.

Computes u[c] = M_tilde[0,0] @ y_rev[0]
             + sum_ij  c2[i,j] (M_tilde[i,j] @ y_rev[j])
             + sum_lk  c3[l,k] (M[l,k,0,0] @ y_rev[k])
             + sum_ijlk c2[i,j] c3[l,k] (M[i,j,l,k] @ y_rev[j+k])

Term 3 streams the 340 MB M tensor; everything else is <1% of the bytes.
Strategy: shard M's leading i axis across 8 cores (3 i-values each,
42.5 MB contiguous per core). On each core, stream the slab through the
tensor engine as a weighted reduction: rhs tiles [128 part, 512 free]
(contiguous HBM), stationary lhsT [128, 64] holding the per-row weights
w[row,p'] = c2[i,j]*c3[l,k]*y_rev[j+k,p'] replicated over 8 p' columns
and 8 row-triples, accumulated into a single PSUM bank [64, 512].
The valid entries sit on the (triple, p'==p) diagonal; the host gathers
them and all-reduces over cores. Small terms 0-2 are computed on host.

Perf structure (fp8e3 mode, measured 53-59 us/core depending on chip
DVFS state, vs 123 us fp32r baseline):
- M quantized to fp8 e3m4 on host (1 B/elem, 10.6 MB/core): the 10.6M-
  term random sum keeps rel err at the per-element sigma (~1.3e-2 with
  bf16 weights + bf16 yc3 consts, deterministic; gate is 2e-2).
- PE streams 135 matmuls x 512 cols at 1 col/cycle @2.4 GHz = 29 us;
  DMA floor 11.1 MB at ~367 GB/s = 30 us — balanced when unthrottled.
- 3 of 18 chunks are offloaded to the otherwise-idle Vector engine as
  8 contiguous scalar_tensor_tensor fused multiply+reduce ops each
  (~760 ns per [128, 576] op; strided patterns are ~9x slower, hence
  the host-side c-major relayout of those chunks). Their [128, 1]
  partials cross-partition-sum via one ones-matmul into "out_d".
- All latency-critical DMA rides the gpsimd queue in explicit FIFO
  order (sync/scalar queues serve at only ~20-40 GB/s and steal port
  bandwidth if loaded): y0 slice, chunk0 (split so the first matmul
  gates on 1/3 of it), then chunks 1-15 as 3-wide groups whose
  partition lines are 13.8 KB (host-relaid; 4.6 KB lines only reach
  ~280 GB/s), chunks 16/17 single at the tail, remaining yc3 slices
  interleaved just ahead of their first consumer. All 5 triples stay
  resident (mg bufs=5) so the slow DVE consumption never blocks the
  DMA stream on a buffer slot.
- Dummy warmup matmuls (+4 fillers after chunk0) burn the PE DVFS ramp
  (~3 us at half clock after idle) on throwaway work.

MODE:
  "fp32"   — exact, PE-bound (fp32 matmul = 4 cycles/row).
  "fp32r"  — TF32-ish matmul (1 cycle/row), rel err ~1.5e-4.
  "bf16"   — M in bf16 (half the HBM bytes), bf16 weights, 1 pass.
             rel err ~6e-3 (harness gate is 2e-2).
  "fp8e3"  — M in fp8 e3m4 (quarter the HBM bytes), bf16 weights
             (mixed-dtype matmul), 1 pass. rel err ~1.1e-2,
             deterministic (fixed seed inputs).
  "bf16x2" — hi/lo bf16 split, 3 matmul passes (hi*hi, hi*lo, lo*hi).
             Same HBM bytes as fp32, rel err ~1e-5, PE 3 cycles/4B.
ONCHIP_W: generate weight tiles on-chip (DVE) from tiny factors instead
of streaming them fused with the M chunks (-11% HBM traffic).
"""

import numpy as np

# ---- problem constants (hardcoded; kernel.py must be self-contained) ----
H, MDIM, C, P = 24, 48, 8, 8
NCORES = 8
IPC = H // NCORES          # i-values per core = 3
NJC = 6                    # j-chunks per i
JCW = MDIM // NJC          # j per chunk = 8
ROWS = JCW * H * MDIM      # rows (of 64 floats) per chunk = 9216
RPP = ROWS // 128          # rows per partition = 72
NT = RPP // 8              # matmuls per chunk = 9
NCHUNK = IPC * NJC         # chunks per core = 18
WCOLS = RPP * 8            # 576 weight columns per chunk
MCOLS = RPP * 64           # 4608 data columns per chunk
NCC = NJC * WCOLS          # consts yc3 columns

MODE = "fp8e3"            # "fp32" | "fp32r" | "bf16" | "fp8e3" | "bf16x2"
ONCHIP_W = True
# chunks computed on the Vector engine via fused multiply+reduce instead
# of the PE, cutting the PE stream. These chunks are relaid c-major on
# host so each of the 8 c-slices is a contiguous [128, 576] block (the
# DVE is ~9x slower on strided access patterns). Their partials return
# via a tiny ones-matmul cross-partition sum in the "out_d" output.
DVE_CHUNKS = (2, 6, 10, 14)

_prog_cache = {}


def _bf16(x):
    import ml_dtypes
    return np.asarray(x).astype(ml_dtypes.bfloat16)


def _build_program():
    import concourse.bass as bass
    import concourse.mybir as mybir
    from concourse.tile import TileContext

    fp32 = mybir.dt.float32
    bf16 = mybir.dt.bfloat16
    mmdt = {"fp32": fp32, "fp32r": mybir.dt.float32r, "bf16": bf16,
            "fp8e3": mybir.dt.float8e3, "bf16x2": bf16}[MODE]
    wdt = bf16 if MODE in ("bf16", "fp8e3") else mmdt
    nc = bass.Bass()

    # fused-chunk column layout (in mmdt elements)
    if MODE == "bf16x2":
        ccols_fused = 2 * WCOLS + 2 * MCOLS     # wh | wl | Mh | Ml
        m_cols = 2 * MCOLS                      # Mh | Ml (onchip variant)
    else:
        ccols_fused = WCOLS + MCOLS             # w | M
        m_cols = MCOLS

    fp16 = mybir.dt.float16
    if ONCHIP_W:
        # chunks jc-major, all M + yc3 traffic on the gpsimd queue in
        # explicit FIFO order (other queues are slow and steal port bw):
        # y0, c0 (split), c1, c2 singles for fast pipeline fill, then
        # triples with 13.8 KB partition lines (~367 GB/s), yc3 slices
        # interleaved just ahead of their first consumer.
        m_sing = nc.dram_tensor("m_sing", [3, 128, m_cols], mmdt,
                                kind="ExternalInput")
        m_trip = nc.dram_tensor("m_trip", [5, 128, IPC * m_cols], mmdt,
                                kind="ExternalInput")
        consts_a = nc.dram_tensor("consts_a", [128, NCHUNK], fp32,
                                  kind="ExternalInput")
        consts_y = nc.dram_tensor("consts_y", [128, NCC], bf16,
                                  kind="ExternalInput")
    else:
        chunks = nc.dram_tensor("chunks", [NCHUNK, 128, ccols_fused], mmdt,
                                kind="ExternalInput")
    out = nc.dram_tensor("out", [64, 512], fp32, kind="ExternalOutput")
    out_d = nc.dram_tensor("out_d", [1, 8 * len(DVE_CHUNKS)], fp32,
                           kind="ExternalOutput")

    with TileContext(nc) as tc:
        with (
            tc.tile_pool(name="consts", bufs=1) as consts,
            tc.tile_pool(name="mpool", bufs=4) as mpool,
            tc.tile_pool(name="wpool", bufs=3) as wpool,
            tc.tile_pool(name="opool", bufs=1) as opool,
            tc.tile_pool(name="psum", bufs=2, space="PSUM") as psum_pool,
        ):
            if ONCHIP_W:
                ca_sb = consts.tile([128, NCHUNK], fp32, tag="ca")
                nc.sync.dma_start(out=ca_sb[:], in_=consts_a[:])
                cy_sb = consts.tile([128, NCC], bf16, tag="cy")
                nc.gpsimd.dma_start(out=cy_sb[:, 0:WCOLS],
                                    in_=consts_y[:, 0:WCOLS])
                # merged slice ranges: fewer DMA_DIRECT2D issues (~680ns
                # of gpsimd engine time each) pull every later transfer in
                y_after = {0: [(1, 3)], 1: [(3, 6)]}

            acc = psum_pool.tile([64, 512], fp32)

            # PE p-state warmup: dummy matmuls during the startup window so
            # the DVFS ramp (half-speed for the first ~3us of PE activity)
            # burns on throwaway work instead of the real stream.
            warm_sb = consts.tile([128, 512], mmdt, tag="warm")
            nc.vector.memset(warm_sb[:], 1.0)
            ones_sb = consts.tile([128, 1], fp32, tag="ones")
            nc.vector.memset(ones_sb[:], 1.0)
            ndc = 8 * len(DVE_CHUNKS)
            dacc = wpool.tile([128, ndc], fp32, tag="dacc", bufs=1)
            dscr = wpool.tile([128, WCOLS], bf16, tag="dscr", bufs=1)
            dscr2 = wpool.tile([128, WCOLS], bf16, tag="dscr2", bufs=1)
            warm_acc = psum_pool.tile([64, 512], fp32, bufs=1)
            for _ in range(7):
                nc.tensor.matmul(warm_acc[:], warm_sb[:, :64], warm_sb[:],
                                 start=True, stop=True)

            n_mm = (NCHUNK - len(DVE_CHUNKS)) * NT
            mm = 0

            assert MODE in ("fp32", "fp32r", "bf16", "fp8e3") and ONCHIP_W
            # pass 1: all DMA issues (gpsimd, explicit FIFO order) and all
            # weight preps (vector, ahead of the DVE chunk work)
            mt_cur = None
            msrcs, wts = [], []
            for ch in range(NCHUNK):
                jc = ch // IPC  # chunks ordered jc-major
                if ch == 0:
                    mt = mpool.tile([128, m_cols], mmdt, tag="ms", bufs=3)
                    # split load: first matmuls gate on the first 1/3
                    nc.gpsimd.dma_start(out=mt[:, :1536],
                                        in_=m_sing[0][:, :1536])
                    nc.gpsimd.dma_start(out=mt[:, 1536:],
                                        in_=m_sing[0][:, 1536:])
                    msrc_full = mt
                elif ch >= 16:
                    mt = mpool.tile([128, m_cols], mmdt, tag="ms", bufs=3)
                    nc.gpsimd.dma_start(out=mt[:], in_=m_sing[ch - 15])
                    msrc_full = mt
                else:
                    sub = (ch - 1) % 3
                    if sub == 0:
                        mt_cur = mpool.tile([128, IPC * m_cols], mmdt,
                                            tag="mg", bufs=5)
                        nc.gpsimd.dma_start(out=mt_cur[:],
                                            in_=m_trip[(ch - 1) // 3])
                    msrc_full = mt_cur[:, sub * m_cols:(sub + 1) * m_cols]
                msrcs.append(msrc_full)
                for s0, s1 in y_after.get(ch, ()):
                    nc.gpsimd.dma_start(
                        out=cy_sb[:, s0 * WCOLS:s1 * WCOLS],
                        in_=consts_y[:, s0 * WCOLS:s1 * WCOLS])
                yc_src = cy_sb[:, jc * WCOLS:(jc + 1) * WCOLS]
                wt = wpool.tile([128, WCOLS], wdt, tag="w", bufs=NCHUNK)
                sc = ca_sb[:, ch:ch + 1]
                if ch == 0:
                    # split weight prep so the first matmul gates on 1/3
                    for k in range(3):
                        nc.vector.tensor_scalar_mul(
                            wt[:, k * 192:(k + 1) * 192],
                            yc_src[:, k * 192:(k + 1) * 192], sc)
                else:
                    nc.vector.tensor_scalar_mul(wt[:], yc_src, sc)
                wts.append(wt)

            # pass 2: PE matmuls, with DVE_CHUNKS offloaded to the vector
            # engine as 8 contiguous fused multiply+reduce ops per chunk
            for ch in range(NCHUNK):
                msrc_full, wt = msrcs[ch], wts[ch]
                if ch in DVE_CHUNKS:
                    di = DVE_CHUNKS.index(ch)
                    for c in range(8):
                        nc.vector.scalar_tensor_tensor(
                            out=(dscr if c % 2 == 0 else dscr2)[:],
                            in0=msrc_full[:, c * WCOLS:(c + 1) * WCOLS],
                            scalar=1.0, in1=wt[:],
                            op0=mybir.AluOpType.bypass,
                            op1=mybir.AluOpType.mult,
                            accum_out=dacc[:, di * 8 + c:di * 8 + c + 1])
                else:
                    for t in range(NT):
                        c0, c1 = 512 * t, 512 * (t + 1)
                        w0, w1 = 64 * t, 64 * (t + 1)
                        nc.tensor.matmul(
                            acc[:], wt[:, w0:w1], msrc_full[:, c0:c1],
                            start=(mm == 0), stop=(mm == n_mm - 1))
                        mm += 1
                if ch == 0:
                    # keep the PE clock hot through the triple-1 gap
                    for _ in range(4):
                        nc.tensor.matmul(warm_acc[:], warm_sb[:, :64],
                                         warm_sb[:], start=True, stop=True)

            # cross-partition sum of the DVE partials via a tiny matmul
            dout_ps = psum_pool.tile([1, ndc], fp32, tag="dout", bufs=1)
            nc.tensor.matmul(dout_ps[:], ones_sb[:, 0:1], dacc[:],
                             start=True, stop=True)
            dout_sb = opool.tile([1, ndc], fp32, tag="outd")
            nc.vector.tensor_copy(dout_sb[:], dout_ps[:])

            out_sb = opool.tile([64, 512], fp32, tag="out")
            nc.vector.tensor_copy(out_sb[:], acc[:])
            nc.sync.dma_start(out=out_d[:], in_=dout_sb[:])
            nc.sync.dma_start(out=out[:], in_=out_sb[:])

    _split_multi_waits(nc, mybir)
    return nc


def _split_multi_waits(nc, mybir):
    """This walrus build encodes at most one sync-wait per instruction
    ("Too many sync wait commands"). Tile emits up to ~2 (slot-release +
    prior-DMA WAW) and ~10 on the final drain. Hoist extra waits onto
    same-engine NoOps that execute immediately before the instruction —
    semantically identical, since sequencer waits are serial anyway."""
    skip = (mybir.InstNoOp, mybir.InstEventSemaphore,
            mybir.InstAllEngineBarrier)
    for fn in nc.m.functions:
        for blk in fn.blocks:
            idx = 0
            while idx < len(blk.instructions):
                inst = blk.instructions[idx]
                si = inst.sync_info
                if (not isinstance(inst, skip) and si is not None
                        and si.on_wait and len(si.on_wait) > 1):
                    waits = list(si.on_wait)
                    si.on_wait = [waits[-1]]
                    for w in waits[:-1]:
                        nop = mybir.InstNoOp(
                            name=nc.get_next_instruction_name(),
                            sync_info=mybir.SyncInfo(on_wait=[w],
                                                     on_update=[]),
                            engine=inst.engine,
                            bass_nofuse=True,
                        )
                        nc.register_instruction(nop)
                        blk.instructions.insert(idx, nop)
                        idx += 1
                idx += 1


def get_program():
    if "nc" not in _prog_cache:
        _prog_cache["nc"] = _build_program()
    return _prog_cache["nc"]


def _weights_and_slabs(y_rev, M, sigma, lambda_e, phi, phi_tilde):
    lam4 = lambda_e ** 0.25
    sig4 = sigma ** 0.25
    c2 = (lam4[:, None] * phi.T).astype(np.float32)        # [H, MDIM] (i,j)
    c3 = (sig4[:, None] * phi_tilde.T).astype(np.float32)  # [H, MDIM] (l,k)
    y = y_rev[:, :, 0].astype(np.float32)                  # [2m, p]

    rows = np.arange(ROWS)
    jl = rows // (H * MDIM)
    lk = rows % (H * MDIM)
    l = lk // MDIM
    kk = lk % MDIM
    jc_idx = np.arange(NJC)[:, None]
    yidx = jc_idx * JCW + jl[None, :] + kk[None, :]        # [NJC, ROWS]
    yc3 = (c3[l, kk][None, :, None] * y[yidx]).astype(np.float32)
    yc3 = yc3.reshape(NJC, 128, WCOLS)
    # partition-major [q, jc*WCOLS+col] so it loads as one DMA
    yc3 = np.ascontiguousarray(yc3.transpose(1, 0, 2).reshape(128, NCC))

    q = np.arange(128)
    c2cols = np.empty((NCORES, 128, NCHUNK), np.float32)
    for core in range(NCORES):
        for il in range(IPC):
            for jc in range(NJC):
                c2cols[core, :, jc * IPC + il] = c2[core * IPC + il,
                                                   jc * JCW + q // 16]
    return yc3, c2cols


def make_core_inputs(y_rev, M, sigma, lambda_e, phi, phi_tilde):
    """Host-side prep of the per-core device inputs for term 3."""
    yc3, c2cols = _weights_and_slabs(y_rev, M, sigma, lambda_e, phi,
                                     phi_tilde)
    yc3_j = yc3.reshape(128, NJC, WCOLS)

    in_maps = []
    for core in range(NCORES):
        slab = M[core * IPC:(core + 1) * IPC]
        slab = np.ascontiguousarray(slab).reshape(NCHUNK, 128, MCOLS)
        # reorder chunks (il, jc) -> jc-major (jc, il) to match the device
        perm = [il * NJC + jc for jc in range(NJC) for il in range(IPC)]
        slab = slab[perm]
        if MODE == "bf16x2":
            sh = _bf16(slab)
            sl = _bf16(slab - sh.astype(np.float32))
        if ONCHIP_W:
            if MODE == "bf16x2":
                mbuf = np.concatenate([sh, sl], axis=2)
            elif MODE == "bf16":
                mbuf = _bf16(slab)
            elif MODE == "fp8e3":
                import ml_dtypes
                mbuf = slab.astype(ml_dtypes.float8_e3m4)
            else:
                mbuf = slab
            mc = mbuf.shape[2]
            # DVE-offloaded chunks: (r, c, p) -> (c, r, p) so each c-slice
            # is a contiguous [128, 576] block for the vector engine
            mbuf = np.ascontiguousarray(mbuf)
            for dch in DVE_CHUNKS:
                mbuf[dch] = (mbuf[dch].reshape(128, RPP, 8, 8)
                             .transpose(0, 2, 1, 3).reshape(128, mc))
            m_trip = mbuf[1:16].reshape(5, 3, 128, mc)
            m_trip = m_trip.transpose(0, 2, 1, 3).reshape(5, 128, 3 * mc)
            in_maps.append({
                "m_sing": np.ascontiguousarray(mbuf[[0, 16, 17]]),
                "m_trip": np.ascontiguousarray(m_trip),
                "consts_a": np.ascontiguousarray(c2cols[core]),
                "consts_y": np.ascontiguousarray(_bf16(yc3)),
            })
        else:
            if MODE == "bf16x2":
                buf = np.empty((NCHUNK, 128, 2 * WCOLS + 2 * MCOLS),
                               _bf16(0.0).dtype)
                for ch in range(NCHUNK):
                    jc = ch % NJC
                    w32 = yc3_j[:, jc] * c2cols[core][:, ch:ch + 1]
                    wh = _bf16(w32)
                    wlv = _bf16(w32 - wh.astype(np.float32))
                    buf[ch, :, :WCOLS] = wh
                    buf[ch, :, WCOLS:2 * WCOLS] = wlv
                    buf[ch, :, 2 * WCOLS:2 * WCOLS + MCOLS] = sh[ch]
                    buf[ch, :, 2 * WCOLS + MCOLS:] = sl[ch]
            else:
                buf = np.empty((NCHUNK, 128, WCOLS + MCOLS), np.float32)
                for ch in range(NCHUNK):
                    jc = ch % NJC
                    buf[ch, :, :WCOLS] = yc3_j[:, jc] * \
                        c2cols[core][:, ch:ch + 1]
                    buf[ch, :, WCOLS:] = slab[ch]
            in_maps.append({"chunks": buf})
    return in_maps


def extract_term3(core_outs):
    """Gather the valid (triple, p-diagonal) entries from the per-core
    [64, 512] PSUM dumps and all-reduce over cores."""
    acc = np.zeros((64, 512), np.float64)
    for o in core_outs:
        acc += o.astype(np.float64)
    e = np.arange(8)[:, None, None]
    p = np.arange(8)[None, :, None]
    c = np.arange(8)[None, None, :]
    return acc[8 * e + p, 64 * e + 8 * c + p].sum((0, 1)).astype(np.float32)


def term3_from_results(results):
    """Full term-3: PE diagonal extraction plus the DVE-offloaded chunk
    partials ([1, 8*len(DVE_CHUNKS)] per core, columns di*8+c)."""
    t = extract_term3([r["out"] for r in results]).astype(np.float64)
    for r in results:
        t = t + r["out_d"].astype(np.float64).reshape(-1, 8).sum(0)
    return t.astype(np.float32)


def host_small_terms(y_rev, M_tilde, M, sigma, lambda_e, phi, phi_tilde):
    lam4 = lambda_e ** 0.25
    sig4 = sigma ** 0.25
    c2 = lam4[:, None] * phi.T
    c3 = sig4[:, None] * phi_tilde.T
    y_m = y_rev[:MDIM]
    u = M_tilde[0, 0] @ y_rev[0]
    u = u + np.einsum("ij,ijcp,jpq->cq", c2, M_tilde, y_m)
    u = u + np.einsum("lk,lkcp,kpq->cq", c3, M[:, :, 0, 0], y_m)
    return u.astype(np.float32)


def kernel(y_rev, M_tilde, M, sigma, lambda_e, phi, phi_tilde):
    from concourse.bass_utils import run_bass_kernel_spmd

    y_rev = np.asarray(y_rev, np.float32)
    M_tilde = np.asarray(M_tilde, np.float32)
    M = np.asarray(M, np.float32)
    sigma = np.asarray(sigma, np.float32)
    lambda_e = np.asarray(lambda_e, np.float32)
    phi = np.asarray(phi, np.float32)
    phi_tilde = np.asarray(phi_tilde, np.float32)

    nc = get_program()
    in_maps = make_core_inputs(y_rev, M, sigma, lambda_e, phi, phi_tilde)
    res = run_bass_kernel_spmd(nc, in_maps, core_ids=list(range(NCORES)))
    term3 = term3_from_results(res.results)

    u = host_small_terms(y_rev, M_tilde, M, sigma, lambda_e, phi, phi_tilde)
    return (u + term3[:, None]).astype(np.float32)

